# revision 7
# baseline (speedup 1.0000x reference)
"""DiffusionGraphConv Trainium2 kernel (8-core SPMD, fp8-DoubleRow design).

Math (per reference, B=32, N=4096, F=128, O=128):
  x = concat(inputs, state)  -> [B, N, F];  x1 = A_s x ; x2 = (2A_s^2 - I) x
  out = sum_m xs_m @ W_m + bias

Reassociation: with B_s = 2 A_s^2 and Y_m = x @ W_m:
  out = x (W0 - W2 - W4) + A_1 Y_1 + B_1 Y_2 + A_2 Y_3 + B_2 Y_4
No Chebyshev dependency chain: all four propagations stream their (dense,
fp8-quantized, power-of-2-scaled) matrix once through the TensorE in
DoubleRow mode (256-row contraction per instruction, 2x bf16 MAC rate).

Sharding: batch across 8 cores (4 batches/core). Host precomputes the
transposed activations x0T (bf16) and the fp8 Y_m tensors, so the device
does nothing but the output-chunk PSUM chains:
  per 512-node chunk rc, per local batch j:
    start matmul (s*(W0-W2-W4))^T @ x0T-chunk  (bf16)
    + 64 fp8 DoubleRow matmuls (Y[k-pair] stationary,
      s*AT_m[k-pair, chunk] moving, accumulating in one PSUM bank)
    ACT evacuation out = psum * (1/s) + bias -> bf16 -> DRAM
"""

import numpy as np
import ml_dtypes

import concourse.bass as bass
import concourse.tile as tile
from concourse import bacc, mybir
from concourse import bass_utils

B, N, D, H, O, S = 32, 4096, 64, 64, 128, 2
F = D + H                    # 128
NCORES = 8
BLOC = B // NCORES           # 4 batches per core
NBLK = N // 128              # 32 n-tiles
NRC = N // 512               # 8 output-node chunks
NPAIR = NBLK // 2            # 16 DoubleRow k-pairs
NM = 4                       # propagation matrices: A1, B1, A2, B2

F32 = mybir.dt.float32
BF16 = mybir.dt.bfloat16
FP8 = mybir.dt.float8e4
DRMODE = mybir.MatmulPerfMode.DoubleRow

_f8 = ml_dtypes.float8_e4m3
_bf = ml_dtypes.bfloat16

_CACHE = {}


def build_nc():
    nc = bacc.Bacc("TRN2", target_bir_lowering=False, debug=False)

    # x0t[j, rc] = x0T_j[f, rc*512:(rc+1)*512]  (x0T_j = [F, N])
    x0t_d = nc.dram_tensor("x0t", [BLOC, NRC, 128, 512], BF16,
                           kind="ExternalInput")
    # yh[j, h, p, i, q] = Y_j[(2h+i)*128+p, q]   (Y_j = [N, 4m*128] fp8)
    yh_d = nc.dram_tensor("yh", [BLOC, NPAIR, 128, 2, 512], FP8,
                          kind="ExternalInput")
    # am[m, rc, g, p, i, q] = (s*AT_m)[(4g+i)*128+p, rc*512+q]
    am_d = nc.dram_tensor("am", [NM, NRC, NPAIR // 2, 128, 4, 512], FP8,
                          kind="ExternalInput")
    v0s_d = nc.dram_tensor("v0s", [128, 128], BF16, kind="ExternalInput")
    bias_d = nc.dram_tensor("bias", [128, 1], F32, kind="ExternalInput")
    sc_d = nc.dram_tensor("sc", [128, 1], F32, kind="ExternalInput")
    # out[j, o, n] = out_core^T per batch
    out_d = nc.dram_tensor("out", [BLOC, 128, N], BF16, kind="ExternalOutput")

    with tile.TileContext(nc) as tc:
        with (
            tc.tile_pool(name="big", bufs=1) as big,
            tc.tile_pool(name="amp", bufs=28) as amp,
            tc.tile_pool(name="stg", bufs=1) as stg,
            tc.tile_pool(name="pso", bufs=8, space=bass.MemorySpace.PSUM) as pso,
        ):
            # ---- resident loads (priority order) ----
            v0s = big.tile([128, 128], BF16, tag="v0s")
            nc.sync.dma_start(v0s[:], v0s_d[:])
            bias_sb = big.tile([128, 1], F32, tag="bias")
            nc.sync.dma_start(bias_sb[:], bias_d[:])
            sc_sb = big.tile([128, 1], F32, tag="sc")
            nc.sync.dma_start(sc_sb[:], sc_d[:])
            x0t = big.tile([128, BLOC, N], BF16, tag="x0t")
            y = big.tile([128, BLOC, NBLK, 512], FP8, tag="y")
            # rc0 x0t chunks (start matmuls), then Y pairs (k-major so the
            # first DR chains unblock early), then the rest of x0t
            for j in range(BLOC):
                nc.sync.dma_start(x0t[:, j, 0:512], x0t_d[j, 0])
            for h in range(NPAIR):
                for j in range(BLOC):
                    nc.sync.dma_start(y[:, j, 2 * h:2 * h + 2, :],
                                      yh_d[j, h])
            for rc in range(1, NRC):
                for j in range(BLOC):
                    nc.sync.dma_start(
                        x0t[:, j, rc * 512:(rc + 1) * 512], x0t_d[j, rc])

            # ---- out-pass ----
            for rc in range(NRC):
                po = [pso.tile([128, 512], F32, tag="po",
                               name=f"po_{rc}_{j}") for j in range(BLOC)]
                for j in range(BLOC):
                    nc.tensor.matmul(
                        po[j][:], v0s[:],
                        x0t[:, j, rc * 512:(rc + 1) * 512],
                        start=True, stop=False)
                for m in range(NM):
                    for g in range(NPAIR // 2):
                        at = amp.tile([128, 4, 512], FP8, tag="am")
                        nc.sync.dma_start(at[:], am_d[m, rc, g])
                        for i2 in (0, 2):
                            last = (m == NM - 1) and (g == NPAIR // 2 - 1) \
                                and (i2 == 2)
                            k0 = 4 * g + i2
                            for j in range(BLOC):
                                nc.tensor.matmul(
                                    po[j][:],
                                    y[:, j, k0:k0 + 2,
                                      m * 128:(m + 1) * 128],
                                    at[:, i2:i2 + 2, :],
                                    start=False, stop=last,
                                    perf_mode=DRMODE)
                for j in range(BLOC):
                    ot = stg.tile([128, 512], BF16, tag="ot", bufs=8)
                    nc.scalar.activation(
                        ot[:], po[j][:],
                        mybir.ActivationFunctionType.Identity,
                        bias=bias_sb[:, 0:1], scale=sc_sb[:, 0:1])
                    nc.sync.dma_start(
                        out_d[j, :, rc * 512:(rc + 1) * 512], ot[:])

    nc.compile()
    return nc


def _dense_at(sup_rows, sup_cols, sup_vals):
    """AT_s dense [S, N, N]: AT[c, r] = sum vals."""
    AT = np.zeros((S, N, N), dtype=np.float32)
    for s in range(S):
        np.add.at(AT[s], (sup_cols[s].astype(np.int64),
                          sup_rows[s].astype(np.int64)),
                  sup_vals[s].astype(np.float32))
    return AT


def _bt_sq(AT):
    """BT_s = 2 * AT_s @ AT_s (== (2 A^2)^T)."""
    try:
        from scipy import sparse
        out = []
        for s in range(S):
            sp = sparse.csr_matrix(AT[s])
            out.append(np.asarray((sp @ sp).todense(), dtype=np.float32) * 2.0)
        return out
    except ImportError:
        return [2.0 * (AT[s] @ AT[s]) for s in range(S)]


def _prep_shared(sup_rows, sup_cols, sup_vals, weight, biases):
    AT = _dense_at(sup_rows, sup_cols, sup_vals)
    BT = _bt_sq(AT)
    mats = [AT[0], BT[0], AT[1], BT[1]]
    mx = max(float(np.abs(m).max()) for m in mats)
    scale = float(2.0 ** np.floor(np.log2(120.0 / mx)))

    # am[m, rc, g, p, i, q] = (s*AT_m)[(4g+i)*128+p, rc*512+q]
    am = np.empty((NM, NRC, NPAIR // 2, 128, 4, 512), dtype=_f8)
    for m in range(NM):
        q = np.asarray(mats[m] * scale, dtype=_f8)
        am[m] = q.reshape(NPAIR // 2, 4, 128, NRC, 512).transpose(
            3, 0, 2, 1, 4)

    W = np.asarray(weight, dtype=np.float32).reshape(F, 5, O)
    v0s = np.ascontiguousarray(
        ((W[:, 0] - W[:, 2] - W[:, 4]) * scale).astype(_bf))
    vcat = np.concatenate([W[:, 1], W[:, 2], W[:, 3], W[:, 4]],
                          axis=1).astype(_bf).astype(np.float32)  # [F, 512]
    bias = np.asarray(biases, dtype=np.float32).reshape(128, 1)
    sc = np.full((128, 1), 1.0 / scale, dtype=np.float32)
    return am, vcat, v0s, bias, sc


def kernel(inputs, state, sup_rows, sup_cols, sup_vals, weight, biases,
           output_size=128, **_ignored):
    inputs = np.asarray(inputs, dtype=np.float32)
    state = np.asarray(state, dtype=np.float32)
    x = np.concatenate(
        [inputs.reshape(B, N, D), state.reshape(B, N, H)], axis=2)  # [B,N,F]

    am, vcat, v0s, bias, sc = _prep_shared(
        np.asarray(sup_rows), np.asarray(sup_cols), np.asarray(sup_vals),
        weight, biases)

    if "nc" not in _CACHE:
        _CACHE["nc"] = build_nc()
    nc = _CACHE["nc"]

    xq = x.astype(_bf).astype(np.float32)   # bf16 activations (as device)
    in_maps = []
    for c in range(NCORES):
        xc = xq[c * BLOC:(c + 1) * BLOC]                     # [BLOC, N, F]
        # x0T per core: [F, BLOC, N] -> chunks [BLOC, NRC, 128, 512]
        xt = np.ascontiguousarray(
            xc.transpose(2, 0, 1).reshape(128, BLOC, NRC, 512).transpose(
                1, 2, 0, 3).astype(_bf))
        # Y_j = bf16(x_j) @ bf16(Vcat) (fp32 accum) -> fp8, pair-tiled
        yh = np.empty((BLOC, NPAIR, 128, 2, 512), dtype=_f8)
        for j in range(BLOC):
            Yj = np.asarray(xc[j] @ vcat, dtype=_f8)         # [N, 512]
            yh[j] = Yj.reshape(NPAIR, 2, 128, 512).transpose(0, 2, 1, 3)
        in_maps.append({
            "x0t": xt, "yh": yh, "am": am, "v0s": v0s, "bias": bias,
            "sc": sc,
        })

    res = None
    for attempt in range(3):
        try:
            res = bass_utils.run_bass_kernel_spmd(
                nc, in_maps, core_ids=list(range(NCORES)), trace=False)
            break
        except Exception:
            if attempt == 2:
                raise
            import time as _time
            _time.sleep(15 * (attempt + 1))

    # reassemble: out_core[j, o, n] -> out[b, n, o]
    outs = np.stack([np.asarray(res.results[c]["out"]).astype(np.float32)
                     for c in range(NCORES)])
    full = outs.transpose(0, 1, 3, 2).reshape(B, N, O)
    return np.ascontiguousarray(full.reshape(B, N * O))


# revision 8
# speedup vs baseline: 1.0225x; 1.0225x over previous
"""DiffusionGraphConv Trainium2 kernel (8-core SPMD, fp8-DoubleRow design).

Math (per reference, B=32, N=4096, F=128, O=128):
  x = concat(inputs, state)  -> [B, N, F];  x1 = A_s x ; x2 = (2A_s^2 - I) x
  out = sum_m xs_m @ W_m + bias

Reassociation: with B_s = 2 A_s^2 and Y_m = x @ W_m:
  out = x (W0 - W2 - W4) + A_1 Y_1 + B_1 Y_2 + A_2 Y_3 + B_2 Y_4
No Chebyshev dependency chain: all four propagations stream their (dense,
fp8-quantized, power-of-2-scaled) matrix once through the TensorE in
DoubleRow mode (256-row contraction per instruction, 2x bf16 MAC rate).

Sharding: batch across 8 cores (4 batches/core). Host precomputes the
transposed activations x0T (bf16) and the fp8 Y_m tensors, so the device
does nothing but the output-chunk PSUM chains:
  per 512-node chunk rc, per local batch j:
    start matmul (s*(W0-W2-W4))^T @ x0T-chunk  (bf16)
    + 64 fp8 DoubleRow matmuls (Y[k-pair] stationary,
      s*AT_m[k-pair, chunk] moving, accumulating in one PSUM bank)
    ACT evacuation out = psum * (1/s) + bias -> bf16 -> DRAM
"""

import numpy as np
import ml_dtypes

import concourse.bass as bass
import concourse.tile as tile
from concourse import bacc, mybir
from concourse import bass_utils

B, N, D, H, O, S = 32, 4096, 64, 64, 128, 2
F = D + H                    # 128
NCORES = 8
BLOC = B // NCORES           # 4 batches per core
NBLK = N // 128              # 32 n-tiles
NRC = N // 512               # 8 output-node chunks
NPAIR = NBLK // 2            # 16 DoubleRow k-pairs
NM = 4                       # propagation matrices: A1, B1, A2, B2

F32 = mybir.dt.float32
BF16 = mybir.dt.bfloat16
FP8 = mybir.dt.float8e4
DRMODE = mybir.MatmulPerfMode.DoubleRow

_f8 = ml_dtypes.float8_e4m3
_bf = ml_dtypes.bfloat16

_CACHE = {}


def build_nc():
    nc = bacc.Bacc("TRN2", target_bir_lowering=False, debug=False)

    # x0t[j, rc] = x0T_j[f, rc*512:(rc+1)*512]  (x0T_j = [F, N])
    x0t_d = nc.dram_tensor("x0t", [BLOC, NRC, 128, 512], BF16,
                           kind="ExternalInput")
    vcat_d = nc.dram_tensor("vcat", [128, 512], BF16, kind="ExternalInput")
    # am[m, rc, g, p, i, q] = (s*AT_m)[(4g+i)*128+p, rc*512+q]
    am_d = nc.dram_tensor("am", [NM, NRC, NPAIR // 2, 128, 4, 512], FP8,
                          kind="ExternalInput")
    v0s_d = nc.dram_tensor("v0s", [128, 128], BF16, kind="ExternalInput")
    bias_d = nc.dram_tensor("bias", [128, 1], F32, kind="ExternalInput")
    sc_d = nc.dram_tensor("sc", [128, 1], F32, kind="ExternalInput")
    # out[j, o, n] = out_core^T per batch
    out_d = nc.dram_tensor("out", [BLOC, 128, N], BF16, kind="ExternalOutput")

    with tile.TileContext(nc) as tc:
        with (
            tc.tile_pool(name="big", bufs=1) as big,
            tc.tile_pool(name="amp", bufs=28) as amp,
            tc.tile_pool(name="stg", bufs=1) as stg,
            tc.tile_pool(name="pst", bufs=4, space=bass.MemorySpace.PSUM) as pst,
            tc.tile_pool(name="pso", bufs=4, space=bass.MemorySpace.PSUM) as pso,
        ):
            # ---- resident loads (priority order) ----
            vcat = big.tile([128, 512], BF16, tag="vcat")
            nc.sync.dma_start(vcat[:], vcat_d[:])
            v0s = big.tile([128, 128], BF16, tag="v0s")
            nc.sync.dma_start(v0s[:], v0s_d[:])
            bias_sb = big.tile([128, 1], F32, tag="bias")
            nc.sync.dma_start(bias_sb[:], bias_d[:])
            sc_sb = big.tile([128, 1], F32, tag="sc")
            nc.sync.dma_start(sc_sb[:], sc_d[:])
            zr = big.tile([128, 1], F32, tag="zr")
            nc.scalar.memzero(zr[:])
            x0t = big.tile([128, BLOC, N], BF16, tag="x0t")
            y = big.tile([128, BLOC, NBLK, 512], FP8, tag="y")
            for rc in range(NRC):
                for j in range(BLOC):
                    nc.sync.dma_start(
                        x0t[:, j, rc * 512:(rc + 1) * 512], x0t_d[j, rc])

            # ---- phase 1: Y-build (k-outer; evac split ACT/DVE halves
            #      so the PSUM recycle latency never paces the PE) ----
            for k in range(NBLK):
                for j in range(BLOC):
                    py = pst.tile([128, 512], F32, tag="py")
                    nc.tensor.matmul(
                        py[:], x0t[:, j, k * 128:(k + 1) * 128], vcat[:],
                        start=True, stop=True)
                    nc.scalar.copy(y[:, j, k, 0:256], py[:, 0:256])
                    nc.vector.tensor_scalar_add(
                        y[:, j, k, 256:512], py[:, 256:512], zr[:, 0:1])

            # ---- phase 2: out-pass ----
            for rc in range(NRC):
                po = [pso.tile([128, 512], F32, tag="po",
                               name=f"po_{rc}_{j}") for j in range(BLOC)]
                for j in range(BLOC):
                    nc.tensor.matmul(
                        po[j][:], v0s[:],
                        x0t[:, j, rc * 512:(rc + 1) * 512],
                        start=True, stop=False)
                for m in range(NM):
                    for g in range(NPAIR // 2):
                        at = amp.tile([128, 4, 512], FP8, tag="am")
                        nc.sync.dma_start(at[:], am_d[m, rc, g])
                        for i2 in (0, 2):
                            last = (m == NM - 1) and (g == NPAIR // 2 - 1) \
                                and (i2 == 2)
                            k0 = 4 * g + i2
                            for j in range(BLOC):
                                nc.tensor.matmul(
                                    po[j][:],
                                    y[:, j, k0:k0 + 2,
                                      m * 128:(m + 1) * 128],
                                    at[:, i2:i2 + 2, :],
                                    start=False, stop=last,
                                    perf_mode=DRMODE)
                for j in range(BLOC):
                    ot = stg.tile([128, 512], BF16, tag="ot", bufs=8)
                    nc.scalar.activation(
                        ot[:], po[j][:],
                        mybir.ActivationFunctionType.Identity,
                        bias=bias_sb[:, 0:1], scale=sc_sb[:, 0:1])
                    nc.sync.dma_start(
                        out_d[j, :, rc * 512:(rc + 1) * 512], ot[:])

    nc.compile()
    return nc


def _dense_at(sup_rows, sup_cols, sup_vals):
    """AT_s dense [S, N, N]: AT[c, r] = sum vals."""
    AT = np.zeros((S, N, N), dtype=np.float32)
    for s in range(S):
        np.add.at(AT[s], (sup_cols[s].astype(np.int64),
                          sup_rows[s].astype(np.int64)),
                  sup_vals[s].astype(np.float32))
    return AT


def _bt_sq(AT):
    """BT_s = 2 * AT_s @ AT_s (== (2 A^2)^T)."""
    try:
        from scipy import sparse
        out = []
        for s in range(S):
            sp = sparse.csr_matrix(AT[s])
            out.append(np.asarray((sp @ sp).todense(), dtype=np.float32) * 2.0)
        return out
    except ImportError:
        return [2.0 * (AT[s] @ AT[s]) for s in range(S)]


def _prep_shared(sup_rows, sup_cols, sup_vals, weight, biases):
    AT = _dense_at(sup_rows, sup_cols, sup_vals)
    BT = _bt_sq(AT)
    mats = [AT[0], BT[0], AT[1], BT[1]]
    mx = max(float(np.abs(m).max()) for m in mats)
    scale = float(2.0 ** np.floor(np.log2(120.0 / mx)))

    # am[m, rc, g, p, i, q] = (s*AT_m)[(4g+i)*128+p, rc*512+q]
    am = np.empty((NM, NRC, NPAIR // 2, 128, 4, 512), dtype=_f8)
    for m in range(NM):
        q = np.asarray(mats[m] * scale, dtype=_f8)
        am[m] = q.reshape(NPAIR // 2, 4, 128, NRC, 512).transpose(
            3, 0, 2, 1, 4)

    W = np.asarray(weight, dtype=np.float32).reshape(F, 5, O)
    v0s = np.ascontiguousarray(
        ((W[:, 0] - W[:, 2] - W[:, 4]) * scale).astype(_bf))
    vcat = np.ascontiguousarray(
        np.concatenate([W[:, 1], W[:, 2], W[:, 3], W[:, 4]],
                       axis=1).astype(_bf))
    bias = np.asarray(biases, dtype=np.float32).reshape(128, 1)
    sc = np.full((128, 1), 1.0 / scale, dtype=np.float32)
    return am, vcat, v0s, bias, sc


def kernel(inputs, state, sup_rows, sup_cols, sup_vals, weight, biases,
           output_size=128, **_ignored):
    inputs = np.asarray(inputs, dtype=np.float32)
    state = np.asarray(state, dtype=np.float32)
    x = np.concatenate(
        [inputs.reshape(B, N, D), state.reshape(B, N, H)], axis=2)  # [B,N,F]

    am, vcat, v0s, bias, sc = _prep_shared(
        np.asarray(sup_rows), np.asarray(sup_cols), np.asarray(sup_vals),
        weight, biases)

    if "nc" not in _CACHE:
        _CACHE["nc"] = build_nc()
    nc = _CACHE["nc"]

    in_maps = []
    for c in range(NCORES):
        # x0T per core: [F, BLOC, N] -> chunks [BLOC, NRC, 128, 512]
        xt = x[c * BLOC:(c + 1) * BLOC].transpose(2, 0, 1)   # [F, BLOC, N]
        xtc = np.ascontiguousarray(
            xt.reshape(128, BLOC, NRC, 512).transpose(1, 2, 0, 3).astype(_bf))
        in_maps.append({
            "x0t": xtc, "am": am, "vcat": vcat, "v0s": v0s, "bias": bias,
            "sc": sc,
        })

    res = None
    for attempt in range(3):
        try:
            res = bass_utils.run_bass_kernel_spmd(
                nc, in_maps, core_ids=list(range(NCORES)), trace=False)
            break
        except Exception:
            if attempt == 2:
                raise
            import time as _time
            _time.sleep(15 * (attempt + 1))

    # reassemble: out_core[j, o, n] -> out[b, n, o]
    outs = np.stack([np.asarray(res.results[c]["out"]).astype(np.float32)
                     for c in range(NCORES)])
    full = outs.transpose(0, 1, 3, 2).reshape(B, N, O)
    return np.ascontiguousarray(full.reshape(B, N * O))
